# revision 20
# baseline (speedup 1.0000x reference)
"""Trainium2 Bass kernel for nn_AttentiveAtlasEncoder (vq_codebook).

Data parallel: B=4096 rows sharded 8 ways (512 rows/core); all params replicated.

Per-core pipeline (all f32):
  - MLP in transposed layout (features on partitions, batch on free):
    xT[128,512] -> gelu(W1)+b1 -> gelu(W2)+b2 -> v^T[32,512] (+bv)
  - scores/softmax/argmax per 128-row tile (batch on partitions)
  - c_bar via PE transpose of router; v_local in both layouts
  - VQ distances via expanded form: dist = -2*v.cb + |cb|^2 + c_offset
    (one K=33 matmul per (tile, 512-chunk); c_offset makes chart segments
    value-disjoint so a single global max_index per tile finds all 8 argmins)
  - argmin: DVE tensor_reduce(min) on PSUM + max_index on SBUF copy
  - codebook gather by index: indirect DMA from a [2048,48] table
    (rows = [codebook | codebook@Ws1 - bs1])
  - z_n MLP: hidden [b,(c,hs)=128] -> PE transpose -> one block-diag matmul
  - hyperbolic distance + vq partial sum reduced on-chip to [1,1]
"""

import os
import sys

import numpy as np

for p in ("/opt/trn_rl_repo", "/root/.axon_site/_ro/trn_rl_repo"):
    if os.path.isdir(p) and p not in sys.path:
        sys.path.insert(0, p)

B, IN, H, D, C, K = 4096, 128, 256, 32, 8, 256
HS = D // 2
NCORES = 8
BS = B // NCORES          # 512 rows per core
NT = BS // 128            # 4 partition tiles per core
CK = C * K                # 2048
EPS = 1e-6
MAX_NORM = 0.99

F32 = np.float32


def _host_prep(inputs):
    """Shared (replicated) device arrays derived from the parameters."""
    W1 = np.ascontiguousarray(inputs["W1"], F32)              # [128,256]
    b1c = np.ascontiguousarray(inputs["b1"].reshape(2, 128).T, F32)   # [128,2]
    W2 = inputs["W2"].astype(F32)
    W2pack = np.ascontiguousarray(
        np.concatenate([W2[0:128, :], W2[128:256, :]], axis=1), F32)  # [128,512]
    b2c = np.ascontiguousarray(inputs["b2"].reshape(2, 128).T, F32)
    Wv = inputs["Wv"].astype(F32)
    Wvpack = np.ascontiguousarray(
        np.concatenate([Wv[0:128, :], Wv[128:256, :]], axis=1), F32)  # [128,64]
    bvrep = np.ascontiguousarray(
        np.broadcast_to(inputs["bv"].astype(F32)[None, :], (128, D)))  # [128,32]
    bvcol = np.ascontiguousarray(inputs["bv"].astype(F32).reshape(D, 1))
    centers = inputs["centers"].astype(F32)
    censcT = np.ascontiguousarray(centers.T / F32(np.sqrt(F32(D))))    # [32,8]
    censb = np.ascontiguousarray(centers, F32)                         # [8,32]

    CB = inputs["codebook"].astype(F32).reshape(CK, D)                 # [2048,32]
    coff = np.repeat(np.arange(C, dtype=F32), K)                       # chart offset
    CBe = np.empty((D + 1, CK), F32)
    CBe[0:D, :] = (F32(-2.0) * CB).T
    CBe[D, :] = (CB * CB).sum(axis=1).astype(F32) + coff               # [33,2048]
    CBe = np.ascontiguousarray(CBe)

    Ws1 = inputs["Ws1"].astype(F32)                                    # [32,16]
    bs1 = inputs["bs1"].astype(F32)
    Gtab = np.zeros((CK, 64), F32)
    Gtab[:, 0:D] = CB
    Gtab[:, D:D + HS] = (CB @ Ws1 - bs1).astype(F32)                   # [2048,64]
    Gtab = np.ascontiguousarray(Gtab)

    Ws2 = inputs["Ws2"].astype(F32)                                    # [16,32]
    W2blk_sb = np.zeros((C * HS, C * D), F32)                          # [128,256]
    for c in range(C):
        W2blk_sb[c * HS:(c + 1) * HS, c * D:(c + 1) * D] = Ws2
    bs2rep = np.ascontiguousarray(
        np.broadcast_to(np.tile(inputs["bs2"].astype(F32), C)[None, :],
                        (128, C * D)))                                 # [128,256]
    # mega-pack: one [128, 1380] tensor for all 128-partition params and one
    # [33, 2048+57] for CBe + the small odd-shaped ones.
    mega = np.concatenate(
        [W1, b1c, W2pack, b2c, Wvpack, bvrep, bs2rep, W2blk_sb], axis=1)
    extra = np.zeros((D + 1, 57), F32)
    extra[0:D, 0:C] = censcT
    extra[0:D, 8:9] = bvcol
    extra[0:D, 9:9 + HS] = Ws1
    extra[0:C, 25:25 + D] = censb
    CBeX = np.ascontiguousarray(np.concatenate([CBe, extra], axis=1))
    return dict(mega=np.ascontiguousarray(mega), CBeX=CBeX, Gtab=Gtab)


def _build_program():
    import concourse.bass as bass
    import concourse.bacc as bacc
    import concourse.tile as tile
    import concourse.mybir as mybir
    from concourse.masks import make_identity

    dt = mybir.dt
    AF = mybir.ActivationFunctionType
    OP = mybir.AluOpType
    AX = mybir.AxisListType

    nc = bacc.Bacc("TRN2", target_bir_lowering=False, debug=False,
                   enable_asserts=False, num_devices=NCORES)

    def din(name, shape, dtype=dt.float32):
        return nc.dram_tensor(name, list(shape), dtype, kind="ExternalInput").ap()

    def dout(name, shape, dtype=dt.float32):
        return nc.dram_tensor(name, list(shape), dtype, kind="ExternalOutput").ap()

    xT_d = din("xT", (128, BS))
    MEGA_W = H + 2 + 2 * H + 2 + 2 * D + D + C * D + C * D   # 1380
    mega_d = din("mega", (128, MEGA_W))
    CBeX_d = din("CBeX", (D + 1, CK + 57))
    G_d = din("Gtab", (CK, 64))

    # z5: [zn | ztx | zgo | cbar | vloc] along the feature dim
    o_z5 = dout("o_z5", (BS, 5 * D))
    o_rtr = dout("o_rtr", (BS, C))
    o_vq = dout("o_vq", (1, 1))
    o_int = dout("o_int", (BS, 2 + C), dt.int32)   # [kch | kco | idx]
    o_zna = dout("o_zna", (BS, C * D))

    S1_d = [nc.dram_tensor(f"scr1_{t}", [128, C], dt.int16,
                           kind="Internal").ap() for t in range(NT)]
    S2_d = [nc.dram_tensor(f"scr2_{t}", [128, C * 8], dt.int16,
                           kind="Internal").ap() for t in range(NT)]

    with tile.TileContext(nc) as tc:
        with (
            tc.tile_pool(name="const", bufs=1) as cst,
            tc.tile_pool(name="work", bufs=1) as wrk,
            tc.tile_pool(name="loop", bufs=3) as lp,
            tc.tile_pool(name="keep", bufs=NT) as kp,
            tc.tile_pool(name="pm", bufs=3, space="PSUM") as pm,
            tc.tile_pool(name="pv", bufs=3, space="PSUM") as pv,
            tc.tile_pool(name="pz", bufs=2, space="PSUM") as pz,
        ):
            mg = cst.tile([128, H + 2 + 2 * H + 2 + 2 * D + D + C * D + C * D],
                          dt.float32, tag="mega")
            nc.sync.dma_start(mg[:], mega_d)
            cbex = cst.tile([D + 1, CK + 57], dt.float32, tag="CBeX")
            nc.sync.dma_start(cbex[:], CBeX_d)

            off = 0
            def seg(w):
                nonlocal off
                s = mg[:, off:off + w]
                off += w
                return s
            W1s = seg(H)
            b1s = seg(2)
            W2s = seg(2 * H)
            b2s = seg(2)
            Wvs = seg(2 * D)
            bvs = seg(D)
            bs2s = seg(C * D)
            W2bs = seg(C * D)
            CBs = cbex[:, 0:CK]
            cTs = cbex[0:D, CK + 0:CK + C]
            bvc = cbex[0:D, CK + 8:CK + 9]
            Ws1s = cbex[0:D, CK + 9:CK + 9 + HS]
            cns = cbex[0:C, CK + 25:CK + 25 + D]

            ident = cst.tile([128, 128], dt.float32, tag="ident")
            make_identity(nc, ident[:])
            ones1 = cst.tile([128, 1], dt.float32, tag="ones1")
            nc.vector.memset(ones1[:], 1.0)
            c256u = cst.tile([128, C], dt.uint32, tag="c256u")
            nc.gpsimd.iota(c256u[:], pattern=[[K, C]], base=0,
                           channel_multiplier=0)
            i8u = cst.tile([128, C], dt.uint32, tag="i8u")
            nc.gpsimd.iota(i8u[:], pattern=[[1, C]], base=0,
                           channel_multiplier=0)

            # ---------------- MLP (transposed layout) ----------------
            xTs = wrk.tile([128, BS], dt.float32, tag="xT")
            nc.sync.dma_start(xTs[:], xT_d)

            f1 = []
            for m in range(2):
                ps = pm.tile([128, BS], dt.float32, tag="pm")
                nc.tensor.matmul(out=ps[:], lhsT=W1s[:, m * 128:(m + 1) * 128],
                                 rhs=xTs[:], start=True, stop=True)
                f = wrk.tile([128, BS], dt.float32, tag=f"f1_{m}")
                nc.scalar.activation(f[:], ps[:], AF.Gelu, bias=b1s[:, m:m + 1])
                f1.append(f)

            f2 = []
            for m in range(2):
                ps = pm.tile([128, BS], dt.float32, tag="pm")
                for k in range(2):
                    nc.tensor.matmul(
                        out=ps[:],
                        lhsT=W2s[:, k * H + m * 128:k * H + (m + 1) * 128],
                        rhs=f1[k][:], start=(k == 0), stop=(k == 1))
                f = wrk.tile([128, BS], dt.float32, tag=f"f2_{m}")
                nc.scalar.activation(f[:], ps[:], AF.Gelu, bias=b2s[:, m:m + 1])
                f2.append(f)

            vps = pv.tile([D, BS], dt.float32, tag="pv")
            for k in range(2):
                nc.tensor.matmul(out=vps[:], lhsT=Wvs[:, k * D:(k + 1) * D],
                                 rhs=f2[k][:], start=(k == 0), stop=(k == 1))
            vTs = wrk.tile([D, BS], dt.float32, tag="vT")
            nc.scalar.activation(vTs[:], vps[:], AF.Copy)
            nc.vector.tensor_scalar_add(vTs[:], vTs[:], bvc[:, 0:1])

            # ------- output staging (one DMA per output at the end) -------
            a_z5 = wrk.tile([128, NT, 5 * D], dt.float32, tag="a_z5")
            a_zna = wrk.tile([128, NT, C * D], dt.float32, tag="a_zna")
            a_int = wrk.tile([128, NT, 2 + C], dt.uint32, tag="a_int")
            a_gidx = wrk.tile([128, NT * C], dt.uint32, tag="a_gidx")

            # ------------- per-tile: router / c_bar / v_local -------------
            cbTs = wrk.tile([D, BS], dt.float32, tag="cbT")
            rsb, vloc, cbss, kchus = [], [], [], []
            for t in range(NT):
                tsl = slice(t * 128, (t + 1) * 128)

                vbp = pv.tile([128, D], dt.float32, tag="pv")
                for k in range(2):
                    nc.tensor.matmul(out=vbp[:], lhsT=f2[k][:, tsl],
                                     rhs=Wvs[:, k * D:(k + 1) * D],
                                     start=(k == 0), stop=(k == 1))
                vb = lp.tile([128, D], dt.float32, tag="vb")
                nc.vector.tensor_add(vb[:], vbp[:], bvs[:, 0:D])

                scp = pv.tile([128, C], dt.float32, tag="pv")
                nc.tensor.matmul(out=scp[:], lhsT=vTs[:, tsl], rhs=cTs[:],
                                 start=True, stop=True)
                scs = lp.tile([128, C], dt.float32, tag="scs")
                nc.scalar.activation(scs[:], scp[:], AF.Copy)
                mx8 = lp.tile([128, 8], dt.float32, tag="mx8")
                nc.vector.max(mx8[:], scs[:])
                nmx = lp.tile([128, 1], dt.float32, tag="nmx")
                nc.vector.tensor_scalar_mul(nmx[:], mx8[:, 0:1], -1.0)
                ex = lp.tile([128, C], dt.float32, tag="ex")
                sume = lp.tile([128, 1], dt.float32, tag="sume")
                nc.scalar.activation(ex[:], scs[:], AF.Exp, bias=nmx[:, 0:1],
                                     accum_out=sume[:])
                rs = lp.tile([128, 1], dt.float32, tag="rs")
                nc.vector.reciprocal(rs[:], sume[:])
                rt = kp.tile([128, C], dt.float32, tag="router")
                nc.vector.tensor_scalar_mul(rt[:], ex[:], rs[:, 0:1])
                rsb.append(rt)

                kch = lp.tile([128, 8], dt.uint32, tag="kch")
                nc.vector.max_index(kch[:], mx8[:], scs[:])
                nc.vector.tensor_copy(a_int[:, t, 0:1], kch[:, 0:1])
                kchus.append(a_int[:, t, 0:1])

                rTp = pv.tile([C, 128], dt.float32, tag="pv")
                nc.tensor.transpose(out=rTp[:], in_=rt[:], identity=ident[:])
                rTs = lp.tile([C, 128], dt.float32, tag="rTs")
                nc.scalar.activation(rTs[:], rTp[:], AF.Copy)

                cTp = pv.tile([D, 128], dt.float32, tag="pv")
                nc.tensor.matmul(out=cTp[:], lhsT=cns[:], rhs=rTs[:],
                                 start=True, stop=True)
                nc.scalar.activation(cbTs[:, tsl], cTp[:], AF.Copy)

                cbp = pv.tile([128, D], dt.float32, tag="pv")
                nc.tensor.matmul(out=cbp[:], lhsT=rTs[:], rhs=cns[:],
                                 start=True, stop=True)
                cbs = a_z5[:, t, 3 * D:4 * D]
                nc.scalar.activation(cbs, cbp[:], AF.Copy)
                cbss.append(cbs)

                vl = a_z5[:, t, 4 * D:5 * D]
                nc.vector.tensor_sub(vl, vb[:], cbs)
                vloc.append(vl)

            # v_local^T and vx = [v_local^T ; ones]
            vx = wrk.tile([D + 1, BS], dt.float32, tag="vx")
            nc.vector.tensor_sub(vx[0:D, :], vTs[:], cbTs[:])
            nc.vector.memset(vx[D:D + 1, :], 1.0)


            # ---------------- distances + argmin ----------------
            zgA = wrk.tile([128, NT * C, 64], dt.float32, tag="zgA")
            gidxs = []
            for t in range(NT):
                tsl = slice(t * 128, (t + 1) * 128)
                dsb = lp.tile([128, CK], dt.float32, tag="dsb")
                msb = lp.tile([128, C], dt.float32, tag="msb")
                for ch in range(4):
                    csl = slice(ch * 512, (ch + 1) * 512)
                    dps = pm.tile([128, 512], dt.float32, tag="pm")
                    nc.tensor.matmul(out=dps[:], lhsT=vx[:, tsl],
                                     rhs=CBs[:, csl], start=True, stop=True)
                    nc.vector.tensor_reduce(
                        out=msb[:, ch * 2:(ch + 1) * 2],
                        in_=dps[:].rearrange("p (c k) -> p c k", k=K),
                        axis=AX.X, op=OP.min)
                    nc.scalar.activation(dsb[:, csl], dps[:], AF.Copy)

                gidx = a_gidx[:, t * C:(t + 1) * C]
                nc.vector.max_index(gidx, msb[:], dsb[:])
                kidx = a_int[:, t, 2:2 + C]
                nc.vector.tensor_sub(kidx, gidx, c256u[:])

                oh8 = lp.tile([128, C], dt.uint32, tag="oh8")
                nc.vector.tensor_tensor(
                    out=oh8[:], in0=i8u[:],
                    in1=kchus[t].to_broadcast([128, C]),
                    op=OP.is_equal)
                ohk = lp.tile([128, C], dt.uint32, tag="ohk")
                nc.vector.tensor_tensor(out=ohk[:], in0=oh8[:], in1=kidx,
                                        op=OP.mult)
                nc.vector.tensor_reduce(out=a_int[:, t, 1:2], in_=ohk[:],
                                        axis=AX.X, op=OP.max)

                # per-tile index re-layout chain + gather (pipelines with
                # the next tile's distance work)
                gx16 = lp.tile([128, C], dt.int16, tag="gx16")
                nc.vector.tensor_copy(gx16[:], gidx)
                nc.sync.dma_start(S1_d[t], gx16[:])
                w16 = lp.tile([16, C * 8], dt.int16, tag="w16")
                nc.sync.dma_start(
                    w16[:].rearrange("r (j g) -> r j g", g=8),
                    S1_d[t].rearrange("(g r) j -> r j g", g=8))
                nc.sync.dma_start(
                    S2_d[t].rearrange("(x r) q -> r x q", x=8),
                    w16[:].unsqueeze(1).to_broadcast([16, 8, C * 8]))
                idxr = lp.tile([128, C * 8], dt.int16, tag="idxr")
                nc.sync.dma_start(idxr[:], S2_d[t])
                nc.gpsimd.dma_gather(
                    out_ap=zgA[:, t * C:(t + 1) * C, :], in_ap=G_d,
                    idxs_ap=idxr[:], num_idxs=128 * C, num_idxs_reg=128 * C,
                    elem_size=64)

            # ---------------- z_n path ----------------
            zgs = [zgA[:, t * C:(t + 1) * C, :] for t in range(NT)]
            for t in range(NT):
                tsl = slice(t * 128, (t + 1) * 128)
                zg = zgs[t]

                h1p = pv.tile([128, HS], dt.float32, tag="pv")
                nc.tensor.matmul(out=h1p[:], lhsT=vx[0:D, tsl], rhs=Ws1s[:],
                                 start=True, stop=True)
                h1s = lp.tile([128, HS], dt.float32, tag="h1s")
                nc.scalar.activation(h1s[:], h1p[:], AF.Copy)

                pre = lp.tile([128, C * HS], dt.float32, tag="pre")
                nc.vector.tensor_tensor(
                    out=pre[:].rearrange("p (c h) -> p c h", h=HS),
                    in0=h1s[:].unsqueeze(1).to_broadcast([128, C, HS]),
                    in1=zg[:, :, D:D + HS],
                    op=OP.subtract)
                hid = lp.tile([128, C * HS], dt.float32, tag="hid")
                nc.scalar.activation(hid[:], pre[:], AF.Gelu)
                hTp = pv.tile([C * HS, 128], dt.float32, tag="pv")
                nc.tensor.transpose(out=hTp[:], in_=hid[:], identity=ident[:])
                hTs = lp.tile([C * HS, 128], dt.float32, tag="hTs")
                nc.scalar.activation(hTs[:], hTp[:], AF.Copy)

                znp = pz.tile([128, C * D], dt.float32, tag="pz")
                nc.tensor.matmul(out=znp[:], lhsT=hTs[:], rhs=W2bs[:],
                                 start=True, stop=True)
                zna = a_zna[:, t, :]
                nc.vector.tensor_add(zna, znp[:], bs2s[:])

                rb = rsb[t][:].unsqueeze(2).to_broadcast([128, C, D])
                znr = lp.tile([128, C * D], dt.float32, tag="znr")
                nc.vector.tensor_tensor(
                    out=znr[:].rearrange("p (c d) -> p c d", d=D),
                    in0=zna.rearrange("p (c d) -> p c d", d=D),
                    in1=rb, op=OP.mult)
                zns = a_z5[:, t, 0:D]
                nc.vector.tensor_reduce(
                    out=zns, in_=znr[:].rearrange("p (c d) -> p d c", d=D),
                    axis=AX.X, op=OP.add)

                zqr = lp.tile([128, C * D], dt.float32, tag="zqr")
                nc.vector.tensor_tensor(
                    out=zqr[:].rearrange("p (c d) -> p c d", d=D),
                    in0=zg[:, :, 0:D], in1=rb, op=OP.mult)
                zqb = lp.tile([128, D], dt.float32, tag="zqb")
                nc.vector.tensor_reduce(
                    out=zqb[:], in_=zqr[:].rearrange("p (c d) -> p d c", d=D),
                    axis=AX.X, op=OP.add)

                tt = lp.tile([128, D], dt.float32, tag="tt")
                nc.vector.tensor_sub(tt[:], vloc[t], zqb[:])
                nc.vector.tensor_sub(a_z5[:, t, D:2 * D], tt[:], zns)

                tg = lp.tile([128, D], dt.float32, tag="tg")
                nc.vector.tensor_add(tg[:], cbss[t], zqb[:])
                nc.vector.tensor_add(a_z5[:, t, 2 * D:3 * D], tg[:], zns)

            # ---------------- vq loss (batched stats) ----------------
            vl2 = wrk.tile([128, NT], dt.float32, tag="vl2")
            sqa = wrk.tile([128, NT * C], dt.float32, tag="sqa")
            xna = wrk.tile([128, NT * C], dt.float32, tag="xna")
            ral = wrk.tile([128, NT * C], dt.float32, tag="ral")
            for t in range(NT):
                scr = lp.tile([128, D], dt.float32, tag="scr")
                nc.scalar.activation(scr[:], vloc[t], AF.Square,
                                     accum_out=vl2[:, t:t + 1])
                nc.vector.tensor_copy(ral[:, t * C:(t + 1) * C], rsb[t][:])

            nrm = wrk.tile([128, NT], dt.float32, tag="nrm")
            nc.scalar.activation(nrm[:], vl2[:], AF.Sqrt)
            nc.vector.tensor_scalar_max(nrm[:], nrm[:], EPS)
            rn = wrk.tile([128, NT], dt.float32, tag="rn")
            nc.vector.reciprocal(rn[:], nrm[:])
            sS = wrk.tile([128, NT], dt.float32, tag="sS")
            nc.vector.tensor_scalar(sS[:], rn[:], MAX_NORM, 1.0,
                                    OP.mult, OP.min)
            s2 = wrk.tile([128, NT], dt.float32, tag="s2")
            nc.vector.tensor_tensor(out=s2[:], in0=sS[:], in1=sS[:], op=OP.mult)
            yn = wrk.tile([128, NT], dt.float32, tag="yn")
            nc.vector.tensor_tensor(out=yn[:], in0=s2[:], in1=vl2[:], op=OP.mult)
            ona = wrk.tile([128, NT], dt.float32, tag="ona")
            nc.vector.tensor_scalar(ona[:], yn[:], -1.0, 1.0, OP.mult, OP.add)

            for t in range(NT):
                vp = lp.tile([128, D], dt.float32, tag="vp")
                nc.vector.tensor_scalar_mul(vp[:], vloc[t], sS[:, t:t + 1])
                dfv = lp.tile([128, C * D], dt.float32, tag="dfv")
                nc.vector.tensor_tensor(
                    out=dfv[:].rearrange("p (c d) -> p c d", d=D),
                    in0=zgs[t][:, :, 0:D],
                    in1=vp[:].unsqueeze(1).to_broadcast([128, C, D]),
                    op=OP.subtract)
                d2v = lp.tile([128, C * D], dt.float32, tag="d2v")
                nc.scalar.activation(d2v[:], dfv[:], AF.Square)
                nc.vector.tensor_reduce(
                    out=sqa[:, t * C:(t + 1) * C],
                    in_=d2v[:].rearrange("p (c d) -> p c d", d=D),
                    axis=AX.X, op=OP.add)
                zq2 = lp.tile([128, C * D], dt.float32, tag="zq2")
                nc.scalar.activation(zq2[:].rearrange('p (c d) -> p c d', d=D), zgs[t][:, :, 0:D], AF.Square)
                nc.vector.tensor_reduce(
                    out=xna[:, t * C:(t + 1) * C],
                    in_=zq2[:].rearrange("p (c d) -> p c d", d=D),
                    axis=AX.X, op=OP.add)

            oxn = wrk.tile([128, NT * C], dt.float32, tag="oxn")
            nc.vector.tensor_scalar(oxn[:], xna[:], -1.0, 1.0, OP.mult, OP.add)
            den = wrk.tile([128, NT * C], dt.float32, tag="den")
            nc.vector.tensor_tensor(
                out=den[:].rearrange("p (t c) -> p t c", c=C),
                in0=oxn[:].rearrange("p (t c) -> p t c", c=C),
                in1=ona[:].unsqueeze(2).to_broadcast([128, NT, C]),
                op=OP.mult)
            nc.vector.tensor_scalar_max(den[:], den[:], EPS)
            rde = wrk.tile([128, NT * C], dt.float32, tag="rde")
            nc.vector.reciprocal(rde[:], den[:])
            arg = wrk.tile([128, NT * C], dt.float32, tag="arg")
            nc.vector.tensor_tensor(out=arg[:], in0=sqa[:], in1=rde[:],
                                    op=OP.mult)
            nc.vector.tensor_scalar(arg[:], arg[:], 2.0, 1.0, OP.mult, OP.add)
            nc.vector.tensor_scalar_max(arg[:], arg[:], 1.0 + EPS)
            ag2 = wrk.tile([128, NT * C], dt.float32, tag="ag2")
            nc.vector.tensor_tensor(out=ag2[:], in0=arg[:], in1=arg[:],
                                    op=OP.mult)
            nc.vector.tensor_scalar_add(ag2[:], ag2[:], -1.0)
            rt2 = wrk.tile([128, NT * C], dt.float32, tag="rt2")
            nc.scalar.activation(rt2[:], ag2[:], AF.Sqrt)
            uu = wrk.tile([128, NT * C], dt.float32, tag="uu")
            nc.vector.tensor_add(uu[:], arg[:], rt2[:])
            dcb = wrk.tile([128, NT * C], dt.float32, tag="dcb")
            nc.scalar.activation(dcb[:], uu[:], AF.Ln)
            dw = wrk.tile([128, NT * C], dt.float32, tag="dw")
            nc.vector.tensor_tensor(out=dw[:], in0=dcb[:], in1=dcb[:],
                                    op=OP.mult)
            nc.vector.tensor_tensor(out=dw[:], in0=dw[:], in1=ral[:],
                                    op=OP.mult)
            prt = wrk.tile([128, 1], dt.float32, tag="prt")
            nc.vector.tensor_reduce(out=prt[:], in_=dw[:], axis=AX.X, op=OP.add)
            vqp = pv.tile([1, 1], dt.float32, tag="pv")
            nc.tensor.matmul(out=vqp[:], lhsT=prt[:], rhs=ones1[:],
                             start=True, stop=True)
            vqs = lp.tile([1, 1], dt.float32, tag="vqs")
            nc.scalar.activation(vqs[:], vqp[:], AF.Copy)
            nc.sync.dma_start(o_vq[:, :], vqs[:])

            # ---------------- batched output DMAs ----------------
            nc.sync.dma_start(o_z5.rearrange("(t p) d -> p t d", p=128), a_z5[:])
            nc.sync.dma_start(o_zna.rearrange("(t p) d -> p t d", p=128), a_zna[:])
            nc.sync.dma_start(o_rtr.rearrange("(t p) c -> p t c", p=128),
                              ral[:].rearrange("p (t c) -> p t c", c=C))
            nc.sync.dma_start(o_int.rearrange("(t p) c -> p t c", p=128),
                              a_int[:].bitcast(dt.int32))

    nc.compile()
    return nc


def kernel(**inputs):
    import time
    from concourse.bass_utils import run_bass_kernel_spmd

    prep = _host_prep(inputs)
    x = inputs["x"].astype(F32)

    t0 = time.time()
    nc = _build_program()
    t1 = time.time()

    in_maps = []
    for c in range(NCORES):
        im = {"xT": np.ascontiguousarray(x[c * BS:(c + 1) * BS, :].T)}
        im.update(prep)
        in_maps.append(im)

    trace = os.environ.get("KERNEL_TRACE", "") == "1"
    res = run_bass_kernel_spmd(nc, in_maps, core_ids=list(range(NCORES)),
                               trace=trace)
    t2 = time.time()
    if os.environ.get("KERNEL_TIMING", "") == "1":
        print(f"[kernel] build+compile {t1 - t0:.1f}s  run {t2 - t1:.1f}s")
    if trace and res.exec_time_ns is not None:
        print(f"HW exec time: {res.exec_time_ns} ns")
        if res.instructions_and_trace:
            print(f"trace: {res.instructions_and_trace[1]}")
    outs = res.results

    def cat(name):
        return np.concatenate([outs[c][name] for c in range(NCORES)], axis=0)

    z5 = cat("o_z5")
    ints = cat("o_int")
    K_chart = ints[:, 0].astype(np.int32)
    K_code = ints[:, 1].astype(np.int32)
    indices = np.ascontiguousarray(ints[:, 2:2 + C]).astype(np.int32)
    z_n = np.ascontiguousarray(z5[:, 0:D])
    z_tex = np.ascontiguousarray(z5[:, D:2 * D])
    z_geo = np.ascontiguousarray(z5[:, 2 * D:3 * D])
    c_bar = np.ascontiguousarray(z5[:, 3 * D:4 * D])
    v_local = np.ascontiguousarray(z5[:, 4 * D:5 * D])
    router = cat("o_rtr")
    z_n_all = cat("o_zna").reshape(B, C, D)
    vq = F32(1.25) * F32(sum(F32(outs[c]["o_vq"][0, 0])
                             for c in range(NCORES))) / F32(B)
    vq_loss = np.asarray(vq, dtype=F32)
    return (K_chart, K_code, z_n, z_tex, router, z_geo, vq_loss, indices,
            z_n_all, c_bar, v_local)
